# revision 6
# baseline (speedup 1.0000x reference)
"""Trainium2 Bass kernel for the 5x5-neighborhood min-L1 loss (nn_NNLoss).

Computation (faithful to the reference):
    gt_pad = pad(ground_truth, rows by nw//2, cols by nh//2, value=-10000)
    norms[b,h,w,s] = sum_c |gt_pad[b,c,h+di,w+dj] - predicted[b,c,h,w]|
                     for s=(di,dj), di in range(nh), dj in range(nw)
    loss = mean over (b,h,w) of min_s norms

Sharding: pure data parallel over the batch dim: 16 images -> 2 per core
across 8 NeuronCores.  Each core returns per-partition partial sums
[128,1]; the host adds them up and divides (the scalar "all-reduce").

v3 layout:
  - the host repacks each core's inputs to bf16 with the padding
    applied: gt -> [H+2*hp, (i c), W+2*wp] (PAD_VAL border), pred ->
    [H, (i c), W].  Row-shifted gt loads then read 128 consecutive
    pre-padded rows = ONE contiguous 3120B DMA descriptor per
    partition (the [i,c,h,w] layout needed 6 512B descriptors per
    partition and ran at ~74 GB/s; this runs near full HBM rate).
    No on-chip casts, no pad memsets, no SWDGE -- gpsimd is idle.
  - partition dim = 128 H-rows (2 row-blocks cover H=256); free dim
    fuses (image, channel, W).
  - all `nw` column shifts of one di: ONE wide DVE sub via an
    overlapping-window AP against a 0-stride broadcast of predicted.
  - |.| on ACT (its only job), split in halves for pipelining.
  - channel sum: 2 DVE adds on strided c-slice views.
  - min over dj: paired-slice merge, then fold into the running m.
Engine budget per core: DVE ~84us (sub 41 + adds 27 + min ~14), ACT
~68us (abs), overlapped across the 10 (block, di) steps.
"""

import os

# The execution path needs the axon PJRT platform; a harness that pins
# JAX_PLATFORMS=cpu would hide the NeuronCores from jax.
if "axon" not in os.environ.get("JAX_PLATFORMS", "axon"):
    os.environ.pop("JAX_PLATFORMS", None)

import numpy as np

B, C, H, W = 16, 3, 256, 256
N_CORES = 8
IPC = B // N_CORES  # images per core
PAD_VAL = -10000.0

_BUILD_CACHE = {}
LAST_EXEC_NS = [None]  # exec_time_ns of the last traced run (for test.py)


def _build(nh, nw):
    """Trace the Bass/Tile program for one core. Returns the Bass object."""
    from contextlib import ExitStack

    import concourse.bacc as bacc
    import concourse.bass as bass  # noqa: F401
    import concourse.tile as tile
    from concourse import mybir
    from concourse.alu_op_type import AluOpType

    f32 = mybir.dt.float32
    f16 = mybir.dt.bfloat16
    Abs = mybir.ActivationFunctionType.Abs

    W_PAD = nh // 2  # pads the W (column) dim -- faithful swap vs torch
    H_PAD = nw // 2  # pads the H (row) dim
    NDI, NDJ = nh, nw  # row / column shift counts
    WP = W + 2 * W_PAD  # padded row width (260)
    HP = H + 2 * H_PAD  # padded row count (260)
    Q = C * IPC  # fused (channel, image) chunks: 6
    FD = Q * W  # 1536
    FDP = Q * WP  # 1560
    SW = IPC * W  # 512: per-(i,w) width of the summed tensor
    assert H % 128 == 0
    NBLK = H // 128

    nc = bacc.Bacc("TRN2", target_bir_lowering=False, debug=False)
    # host-repacked layouts (see kernel()): contiguous per-row loads
    pred_d = nc.dram_tensor("predicted", [H, Q, W], f16, kind="ExternalInput")
    gt_d = nc.dram_tensor("ground_truth", [HP, Q, WP], f16, kind="ExternalInput")
    # [+I | -I] 128x128 stationaries for the TensorE subtract path
    st_d = nc.dram_tensor("stats", [128, 256], f16, kind="ExternalInput")
    out_d = nc.dram_tensor("partials", [128, 1], f32, kind="ExternalOutput")

    import bass_rust as _br

    def strided(ap, levels, extra_offset=0):
        """Hand-built free-dim AP on an existing [128, N] view (keeps the
        partition level and base offset)."""
        c = ap.copy()
        c.ap = _br.VecI64Pair([list(ap.ap[0])] + [list(l) for l in levels])
        if extra_offset:
            c.offset = c.offset + extra_offset
        return c

    G = NDJ  # all column shifts merged into one wide instruction group

    with tile.TileContext(nc) as tc, ExitStack() as ctx:
        p_pool = ctx.enter_context(tc.tile_pool(name="pred", bufs=2))
        g_pool = ctx.enter_context(tc.tile_pool(name="gsel", bufs=4))
        d_pool = ctx.enter_context(tc.tile_pool(name="d", bufs=3))
        s_pool = ctx.enter_context(tc.tile_pool(name="s", bufs=2))
        t_pool = ctx.enter_context(tc.tile_pool(name="t", bufs=2))
        m_pool = ctx.enter_context(tc.tile_pool(name="m", bufs=2))
        r_pool = ctx.enter_context(tc.tile_pool(name="r", bufs=1))

        r_tiles = []
        for b in range(NBLK):
            h0 = 128 * b

            # ---- predicted: one contiguous bf16 DMA ----
            pt = p_pool.tile([128, FD], f16, tag=f"pred{b}")
            nc.sync.dma_start(
                pt.rearrange("p (q w) -> p q w", q=Q),
                pred_d.ap()[h0 : h0 + 128],
            )
            # broadcast view: [p, G(stride 0), Q, W]
            ptb = strided(pt[:, :], [[0, G], [W, Q], [1, W]])

            m = [None]

            def emit_group(g0, dj0, g, m=m, ptb=ptb, b=b):
                """Sub/abs/sum/min for dj in [dj0, dj0+g) of one di."""
                gt_op = strided(g0[:, :], [[1, g], [WP, Q], [1, W]], dj0)
                dG = d_pool.tile([128, g * FD], f16, tag="d")
                d_out = strided(dG[:, :], [[FD, g], [W, Q], [1, W]])
                pb = strided(ptb, [[0, g], [W, Q], [1, W]])
                nc.vector.tensor_sub(d_out, gt_op, pb)
                # |d| on ACT, two halves so the adds can start sooner
                half = (g // 2) * FD
                if half:
                    nc.scalar.activation(dG[:, 0:half], dG[:, 0:half], Abs)
                nc.scalar.activation(dG[:, half : g * FD], dG[:, half : g * FD], Abs)
                # channel sum: chunks are img-major (q = i*C + c), so the
                # c-slices are [g, IPC, W] strided views at offset c*W
                CW = C * W
                dc = [
                    strided(dG[:, :], [[FD, g], [CW, IPC], [1, W]], c * W)
                    for c in range(C)
                ]
                s01 = s_pool.tile([128, g * SW], f16, tag="s01")
                s01v = strided(s01[:, :], [[SW, g], [W, IPC], [1, W]])
                nc.vector.tensor_add(s01v, dc[0], dc[1])
                sG = s_pool.tile([128, g * SW], f16, tag="sG")
                sGv = strided(sG[:, :], [[SW, g], [W, IPC], [1, W]])
                nc.vector.tensor_add(sGv, s01v, dc[2])

                # ---- min over the g dj-slices, pair-merged ----
                npairs = g // 2
                if npairs:
                    u = t_pool.tile([128, npairs * SW], f16, tag="u")
                    in0 = strided(sG[:, :], [[2 * SW, npairs], [1, SW]])
                    in1 = strided(sG[:, :], [[2 * SW, npairs], [1, SW]], SW)
                    uo = strided(u[:, :], [[SW, npairs], [1, SW]])
                    nc.vector.tensor_tensor(uo, in0, in1, AluOpType.min)
                    v = u[:, 0:SW]
                    for k in range(1, npairs):
                        vn = t_pool.tile([128, SW], f16, tag="v")
                        nc.vector.tensor_tensor(
                            vn, v, u[:, k * SW : (k + 1) * SW], AluOpType.min
                        )
                        v = vn
                else:
                    v = None
                odd = sG[:, (g - 1) * SW : g * SW] if g % 2 else None

                terms = [x for x in (v, odd) if x is not None]
                if m[0] is None:
                    mt = m_pool.tile([128, SW], f16, tag=f"m{b}")
                    m[0] = mt
                    if len(terms) == 2:
                        nc.vector.tensor_tensor(m[0], terms[0], terms[1], AluOpType.min)
                    else:
                        nc.vector.tensor_copy(m[0], terms[0])
                else:
                    for tm in terms:
                        nc.vector.tensor_tensor(m[0], m[0], tm, AluOpType.min)

            for di in range(NDI):
                # tile row p holds gt_pad row (h0 + p + di): 128
                # consecutive pre-padded rows, one descriptor/partition
                g0 = g_pool.tile([128, FDP], f16, tag="g")
                nc.sync.dma_start(
                    g0.rearrange("p (q w) -> p q w", q=Q),
                    gt_d.ap()[h0 + di : h0 + di + 128],
                )
                first = b == 0 and di == 0
                last = b == NBLK - 1 and di == NDI - 1
                if (first or last) and G >= 4:
                    # split the pipeline-edge steps so the ACT/adds chain
                    # starts earlier (head) / drains sooner (tail)
                    gh = G // 2
                    emit_group(g0, 0, gh)
                    emit_group(g0, gh, G - gh)
                else:
                    emit_group(g0, 0, G)
            m = m[0]

            r = r_pool.tile([128, 1], f32, tag=f"r{b}")
            nc.vector.tensor_reduce(r, m, mybir.AxisListType.X, AluOpType.add)
            r_tiles.append(r)

        tot = r_tiles[0]
        for b in range(1, NBLK):
            nxt = r_pool.tile([128, 1], f32, tag=f"tot{b}")
            nc.vector.tensor_add(nxt, tot, r_tiles[b])
            tot = nxt
        nc.sync.dma_start(out_d.ap()[:, :], tot)

    nc.compile()
    return nc


def _get_nc(nh, nw):
    key = (nh, nw)
    if key not in _BUILD_CACHE:
        _BUILD_CACHE[key] = _build(nh, nw)
    return _BUILD_CACHE[key]


def _setup_trace():
    """Register the axon NTFF profile hook (the image's antenv lacks
    axon_hooks) and stub the artifact upload so trace=True works."""
    import sys
    import types

    from concourse import bass_utils

    try:
        import antenv.axon_hooks  # noqa: F401
    except ImportError:
        try:
            import trn_agent_boot.trn_boot as tb

            hook = tb._ntff_profile_via_ctypes("/opt/axon/libaxon_pjrt.so")
            mod = types.ModuleType("antenv.axon_hooks")
            mod.get_axon_ntff_profile_hook = lambda: hook
            sys.modules["antenv.axon_hooks"] = mod
        except Exception as e:  # profiling is best-effort
            print(f"ntff hook setup failed: {e}")
            return False
    bass_utils.upload_artifacts = lambda tmpdir: f"local:{tmpdir}"
    return True


def _repack(pred, gt, nh, nw):
    """Per-core host repack: bf16, (i,c) fused, gt pre-padded.

    pred [IPC,C,H,W] -> [H, IPC*C, W]
    gt   [IPC,C,H,W] -> [H+2*hp, IPC*C, W+2*wp] with PAD_VAL border
    """
    import ml_dtypes

    bf16 = ml_dtypes.bfloat16
    wp = nh // 2
    hp = nw // 2
    Q = IPC * C
    p = np.ascontiguousarray(
        pred.reshape(Q, H, W).transpose(1, 0, 2).astype(bf16)
    )
    g = np.full((H + 2 * hp, Q, W + 2 * wp), PAD_VAL, dtype=bf16)
    g[hp : hp + H, :, wp : wp + W] = gt.reshape(Q, H, W).transpose(1, 0, 2)
    return p, np.ascontiguousarray(g)


def kernel(predicted, ground_truth, nh=5, nw=5):
    from concourse import bass_utils

    nh, nw = int(nh), int(nw)
    pred = np.asarray(predicted, dtype=np.float32)
    gt = np.asarray(ground_truth, dtype=np.float32)
    assert pred.shape == (B, C, H, W) and gt.shape == (B, C, H, W)

    nc = _get_nc(nh, nw)
    in_maps = []
    for k in range(N_CORES):
        p, g = _repack(
            pred[k * IPC : (k + 1) * IPC], gt[k * IPC : (k + 1) * IPC], nh, nw
        )
        in_maps.append({"predicted": p, "ground_truth": g})
    trace = bool(int(os.environ.get("NNLOSS_TRACE", "0")))
    if trace:
        trace = _setup_trace()
    res = bass_utils.run_bass_kernel_spmd(
        nc, in_maps, list(range(N_CORES)), trace=trace
    )
    LAST_EXEC_NS[0] = res.exec_time_ns
    total = 0.0
    for r in res.results:
        total += float(np.asarray(r["partials"], dtype=np.float64).sum())
    return np.float32(total / (B * H * W))
